# revision 17
# baseline (speedup 1.0000x reference)
"""ChirpTokenizer Trainium2 kernel.

Math: the reference pipeline (hann window -> per-chirp-rate warp resample
with linear interpolation + jacobian -> rFFT over the warped axis) is
linear in x for each chirp rate d.  It therefore collapses into a single
matmul per d:

    out[b, w, d, f] = sum_k x[b, 512*w + k] * G_d[k, f]

where G_d = diag(hann) @ A_d @ F, A_d is the (K x K_TAU) sparse
interpolation/jacobian matrix (2 nnz per column) and F the rFFT matrix.
Since the input is real, Im(X[0]) = Im(X[1024]) = 0, so the packed real
G_d is (1024 x 2048): [Re f=0..1024 | Im f=1..1023].

G_d depends only on dlnf (16 floats); it is built on the host with a
sparse scatter + FFT (cheap) and shipped to the device.  The device does
pure TensorE work: for each core, 2 chirp rates x (2048 rows x 1024 k x
2048 f) fp16 matmuls, PSUM-accumulated over k (fp16 keeps full rel err
~3.5e-4 since products accumulate in fp32 PSUM; it streams faster than
fp32/fp32r through the PE and halves all transfers).

Sharding: D=16 chirp rates over 8 cores (2 per core); frames replicated.
"""

import numpy as np

K = 1024
HOP = 512
K_TAU = 2048
FK = K_TAU // 2 + 1  # 1025
B = 4
N = 262144
D = 16
NWIN = (N - K) // HOP + 1  # 511
NCORES = 8
DPC = D // NCORES  # 2 chirp rates per core
WPAD = 512  # pad 511 windows -> 512 per batch element
ROWS_PAD = B * WPAD  # 2048

_NC_CACHE = {}


def _warp_grid_impl(dlnf):
    """Replicate the reference's f32 warp-grid computation bit-for-bit.

    Runs via jax on CPU: the grid has a 1/beta cancellation that amplifies
    1-ulp exp/log1p differences into ~1e-3-sample index shifts, so the
    exact XLA-CPU op implementations matter.
    """
    import jax.numpy as jnp

    beta = 2.0 * dlnf
    tau = 2.0 * jnp.arange(K_TAU, dtype=jnp.float32) / K_TAU - 1.0
    small = jnp.abs(beta) < 1e-8
    beta_safe = jnp.where(small, 1e-8, beta)
    e2b = jnp.exp(2.0 * beta_safe)

    t_source = (
        jnp.log1p((tau[None, :] + 1.0) * 0.5 * (e2b[:, None] - 1.0))
        / beta_safe[:, None]
        - 1.0
    )
    t_source = jnp.where(small[:, None], tau[None, :], t_source)

    tau_mid = 2.0 * (K_TAU // 2) / K_TAU - 1.0  # = 0.0
    t_mid = jnp.log1p((tau_mid + 1.0) * 0.5 * (e2b - 1.0)) / beta_safe - 1.0
    t_mid = jnp.where(small, tau_mid, t_mid)

    jac = jnp.exp(-beta_safe[:, None] * (t_source - t_mid[:, None]))
    jac = jnp.where(small[:, None], 1.0, jac)

    idx = (K / 2.0) * (t_source + 1.0)
    idx_lo = jnp.clip(jnp.floor(idx).astype(jnp.int32), 0, K - 2)
    frac = idx - idx_lo.astype(jnp.float32)
    return idx_lo, frac, jac


def _warp_grid_np(dlnf):
    """Numpy fallback (used only if no jax CPU backend is available)."""
    f32 = np.float32
    beta = (f32(2.0) * dlnf).astype(f32)
    tau = (f32(2.0) * np.arange(K_TAU, dtype=f32) / f32(K_TAU) - f32(1.0)).astype(f32)
    small = np.abs(beta) < f32(1e-8)
    beta_safe = np.where(small, f32(1e-8), beta).astype(f32)
    e2b = np.exp((f32(2.0) * beta_safe).astype(f32)).astype(f32)
    t_source = (
        np.log1p((tau[None, :] + f32(1.0)) * f32(0.5) * (e2b[:, None] - f32(1.0)))
        / beta_safe[:, None]
        - f32(1.0)
    ).astype(f32)
    t_source = np.where(small[:, None], tau[None, :], t_source).astype(f32)
    t_mid = (
        np.log1p(f32(0.5) * (e2b - f32(1.0))) / beta_safe - f32(1.0)
    ).astype(f32)
    t_mid = np.where(small, f32(0.0), t_mid).astype(f32)
    jac = np.exp(-beta_safe[:, None] * (t_source - t_mid[:, None])).astype(f32)
    jac = np.where(small[:, None], f32(1.0), jac)
    idx = (f32(K / 2.0) * (t_source + f32(1.0))).astype(f32)
    idx_lo = np.clip(np.floor(idx).astype(np.int32), 0, K - 2)
    frac = (idx - idx_lo.astype(f32)).astype(f32)
    return idx_lo, frac, jac


def _warp_grid(dlnf: np.ndarray):
    dlnf = np.asarray(dlnf, np.float32)
    try:
        import jax

        if "warp_jit" not in _NC_CACHE:
            cpu = jax.local_devices(backend="cpu")[0]
            _NC_CACHE["warp_jit"] = jax.jit(_warp_grid_impl, device=cpu)
        idx_lo, frac, jac = _NC_CACHE["warp_jit"](dlnf)
        return np.asarray(idx_lo), np.asarray(frac), np.asarray(jac)
    except Exception:
        return _warp_grid_np(dlnf)


def _build_g(dlnf: np.ndarray) -> np.ndarray:
    """(D,) f32 -> (D, 8, 128, 2048) fp16: packed DFT-of-resample matrices."""
    f32 = np.float32
    dlnf = np.asarray(dlnf, f32)
    idx_lo, frac, jac = _warp_grid(dlnf)

    # fold the hann window (a function of the source row k) into the
    # interpolation weights so no extra pass over G is needed
    n = np.arange(K, dtype=np.float64)
    hann = (0.5 - 0.5 * np.cos(2.0 * np.pi * n / K)).astype(f32)
    lo = idx_lo.ravel()
    w_lo = ((f32(1.0) - frac) * jac).ravel() * hann[lo]
    w_hi = (frac * jac).ravel() * hann[lo + 1]

    # A[d, k, t]: sparse scatter (indices are unique — lo vs lo+1 never
    # collide for the same t, and t differs otherwise)
    A = np.zeros((D, K, K_TAU), f32)
    d_idx = np.repeat(np.arange(D), K_TAU)
    t_idx = np.tile(np.arange(K_TAU), D)
    A[d_idx, lo, t_idx] = w_lo
    A[d_idx, lo + 1, t_idx] = w_hi

    try:
        from scipy.fft import rfft as _rfft

        W = _rfft(A, axis=-1, workers=-1)  # (D, K, FK) complex64
    except ImportError:
        W = np.fft.rfft(A, axis=-1)

    G = np.empty((D, K, 2048), np.float16)
    G[:, :, :FK] = W.real
    G[:, :, FK:] = W.imag[:, :, 1:1024]
    return np.ascontiguousarray(G.reshape(D, 8, 128, 2048))


def _build_frames_t(x: np.ndarray) -> np.ndarray:
    """(B, N) f32 -> (8, 128, ROWS_PAD) fp16 transposed overlapped frames.

    ft[kc, i, b*512 + w] = x[b, 512*w + 128*kc + i]  (w < 511; w = 511 zero)
    """
    ft = np.zeros((K, ROWS_PAD), np.float16)
    for b in range(B):
        frames = np.lib.stride_tricks.as_strided(
            x[b], shape=(NWIN, K), strides=(HOP * 4, 4)
        )
        ft[:, b * WPAD : b * WPAD + NWIN] = frames.T.astype(np.float16)
    return np.ascontiguousarray(ft.reshape(8, 128, ROWS_PAD))


def _get_nc():
    if "nc" in _NC_CACHE:
        return _NC_CACHE["nc"]
    import concourse.bacc as bacc
    import concourse.mybir as mybir
    from concourse import tile

    nc = bacc.Bacc("TRN2", target_bir_lowering=False, debug=False, num_devices=NCORES)
    ft_d = nc.dram_tensor(
        "ft", [8, 128, ROWS_PAD], mybir.dt.float16, kind="ExternalInput"
    )
    g_d = nc.dram_tensor(
        "g", [DPC, 8, 128, 2048], mybir.dt.float16, kind="ExternalInput"
    )
    out_d = nc.dram_tensor(
        "out", [DPC, ROWS_PAD, 2048], mybir.dt.float16, kind="ExternalOutput"
    )

    with tile.TileContext(nc) as tc:
        with (
            tc.tile_pool(name="ftp", bufs=8) as ftp,
            tc.tile_pool(name="gp", bufs=16) as gp,
            tc.tile_pool(name="op", bufs=4) as op,
            tc.tile_pool(name="pp", bufs=8, space="PSUM") as pp,
        ):
            ftt = []
            for kc in range(8):
                t = ftp.tile(
                    [128, ROWS_PAD], mybir.dt.float16, name=f"ft{kc}", tag="ft"
                )
                nc.sync.dma_start(t[:], ft_d[kc])
                ftt.append(t)
            for d in range(DPC):
                gtt = []
                for kc in range(8):
                    t = gp.tile(
                        [128, 2048], mybir.dt.float16, name=f"g{d}_{kc}", tag="g"
                    )
                    nc.sync.dma_start(t[:], g_d[d, kc])
                    gtt.append(t)
                for m in range(16):
                    ost = op.tile(
                        [128, 2048], mybir.dt.float16, name=f"o{d}_{m}", tag="o"
                    )
                    pss = [
                        pp.tile(
                            [128, 512],
                            mybir.dt.float32,
                            name=f"p{d}_{m}_{nn}",
                            tag="p",
                        )
                        for nn in range(4)
                    ]
                    # k outer / n inner: the first matmuls only need the first
                    # ft/g k-chunks, so compute starts while later chunks are
                    # still in flight (removes the 16MB startup DMA bubble).
                    for kc in range(8):
                        for nn in range(4):
                            nc.tensor.matmul(
                                pss[nn][:],
                                ftt[kc][:, 128 * m : 128 * (m + 1)],
                                gtt[kc][:, 512 * nn : 512 * (nn + 1)],
                                start=(kc == 0),
                                stop=(kc == 7),
                            )
                    for nn in range(4):
                        nc.vector.tensor_copy(
                            ost[:, 512 * nn : 512 * (nn + 1)], pss[nn][:]
                        )
                    nc.sync.dma_start(out_d[d, 128 * m : 128 * (m + 1), :], ost[:])
    nc.compile()
    _NC_CACHE["nc"] = nc
    return nc


def _get_runner():
    """Build (once) a sharded jitted callable over the 8 cores.

    Mirrors the multi-core tail of bass2jax.run_bass_via_pjrt, but caches
    the jitted function so repeat kernel() calls don't re-trace/re-compile.
    Returns (fn, in_names, out_names, out_shapes_dtypes).
    """
    if "runner" in _NC_CACHE:
        return _NC_CACHE["runner"]
    import jax
    import concourse.mybir as mybir
    from concourse import bass2jax
    from jax.sharding import Mesh, PartitionSpec
    from jax.experimental.shard_map import shard_map

    nc = _get_nc()
    bass2jax.install_neuronx_cc_hook()

    partition_name = (
        nc.partition_id_tensor.name if nc.partition_id_tensor is not None else None
    )
    in_names = []
    out_names = []
    out_avals = []
    for alloc in nc.m.functions[0].allocations:
        if not isinstance(alloc, mybir.MemoryLocationSet):
            continue
        name = alloc.memorylocations[0].name
        if alloc.kind == "ExternalInput":
            if name != partition_name:
                in_names.append(name)
        elif alloc.kind == "ExternalOutput":
            shape = tuple(alloc.tensor_shape)
            dtype = mybir.dt.np(alloc.dtype)
            out_names.append(name)
            out_avals.append(jax.core.ShapedArray(shape, dtype))
    n_params = len(in_names)
    n_outs = len(out_names)
    all_names = list(in_names) + list(out_names)
    if partition_name is not None:
        all_names.append(partition_name)
    all_names = tuple(all_names)

    def _body(*args):
        operands = list(args)
        if partition_name is not None:
            operands.append(bass2jax.partition_id_tensor())
        outs = bass2jax._bass_exec_p.bind(
            *operands,
            out_avals=tuple(out_avals),
            in_names=all_names,
            out_names=tuple(out_names),
            lowering_input_output_aliases=(),
            sim_require_finite=True,
            sim_require_nnan=True,
            nc=nc,
        )
        return tuple(outs)

    devices = jax.devices()[:NCORES]
    mesh = Mesh(np.asarray(devices), ("core",))
    # ft is identical on every core: pass it replicated (one wire transfer)
    # instead of 8x-concatenated; every other operand is sharded on axis 0.
    in_specs = tuple(
        PartitionSpec() if name == "ft" else PartitionSpec("core")
        for name in in_names
    ) + (PartitionSpec("core"),) * n_outs
    out_specs = (PartitionSpec("core"),) * n_outs
    fn = jax.jit(
        shard_map(
            _body, mesh=mesh, in_specs=in_specs, out_specs=out_specs, check_rep=False
        ),
        donate_argnums=tuple(range(n_params, n_params + n_outs)),
        keep_unused=True,
    )
    runner = (fn, in_names, out_names, [(a.shape, a.dtype) for a in out_avals], mesh)
    _NC_CACHE["runner"] = runner
    return runner


def kernel(x: np.ndarray, dlnf: np.ndarray) -> np.ndarray:
    x = np.ascontiguousarray(np.asarray(x, dtype=np.float32))
    dlnf = np.asarray(dlnf, dtype=np.float32)

    G = _build_g(dlnf)  # (D, 8, 128, 2048)
    FT = _build_frames_t(x)  # (8, 128, ROWS_PAD)

    fn, in_names, out_names, out_sd, _mesh = _get_runner()
    full_in = {"ft": FT, "g": np.ascontiguousarray(G.reshape(NCORES * DPC, 8, 128, 2048))}
    concat_in = [full_in[name] for name in in_names]
    concat_zeros = [
        np.zeros((NCORES * s[0], *s[1:]), dt) for (s, dt) in out_sd
    ]
    out_arrs = fn(*concat_in, *concat_zeros)
    o_all = np.asarray(out_arrs[out_names.index("out")]).reshape(
        NCORES, DPC, ROWS_PAD, 2048
    )

    # o5[d, b, w, :] with w < NWIN valid; assemble into interleaved complex64
    # via a float32 view so the fp16->f32 cast fuses into the strided copies
    o5 = o_all.reshape(D, B, WPAD, 2048)[:, :, :NWIN, :]
    out_f = np.empty((B, NWIN, D, FK, 2), np.float32)
    out_f[..., 0] = np.transpose(o5[:, :, :, :FK], (1, 2, 0, 3))
    out_f[:, :, :, 0, 1] = 0.0
    out_f[:, :, :, 1024, 1] = 0.0
    out_f[:, :, :, 1:1024, 1] = np.transpose(o5[:, :, :, FK:], (1, 2, 0, 3))
    return out_f.view(np.complex64)[..., 0]
